# revision 55
# baseline (speedup 1.0000x reference)
"""Trainium2 Bass kernel: batched masked sparse attention.

Problem (per batch element b of 8):
    logits = q @ k.T / sqrt(D) - |i - j|                    [S, S]
    mask   = rep_i * rep_j * strict_lower_tri               (rep = first-L valid)
    attn   = masked_softmax(logits, mask)  (mask-multiply semantics)
    out    = attn @ v
Returns (out [B,S,D], attn [B,S,S]).

Strategy: batch-parallel SPMD — one batch element per NeuronCore, 8 cores,
no collectives.  Per core:
  * Only the strictly-lower-triangular region of attn is ever nonzero
    (valid rows i < L imply j < i < L, so the column mask is redundant);
    the upper triangle is left to the runtime's zero-initialized output.
  * No row-max subtraction: valid logits <= qk/sqrt(D) (distance bias is
    negative there), exp can't overflow, and softmax is shift-invariant,
    so e = exp(logits) directly; row sums come free from the ACT
    accumulator.  Invalid rows are zeroed via recip = rep_i/(Z + 1e-20).
  * QK^T and the -|i-j| distance bias are both TensorE matmuls (fp32r
    fast path, 1 cycle/row; the bias via a K=128 zero-padded matmul
    against iota operands) accumulated in PSUM.
  * PV runs on PE-transposed attn blocks: out^T[d, i] accumulates
    v_j^T @ attn^T[j, i] over j, then a final PE transpose restores [i,d].
  * Software pipelining: each 4-block group's PV work is interleaved
    into the NEXT group's phaseA emission (per-engine execution is
    in-order, so this keeps TensorE from stalling on the softmax chain);
    the final group's PV is split into two half-width accumulations so
    half of it overlaps the last phaseA blocks.
  * Host-side prep inside kernel(): q/k are pre-transposed and, like v,
    pre-rounded to fp32r (11 mantissa bits); iota/identity/tril constant
    tiles are shipped as extra NEFF inputs.  GpSimd is kept out of the
    pipeline entirely (its SBUF port is shared with DVE).
"""

import os
import sys

import numpy as np

for _p in ("/opt/trn_rl_repo", "/root/.axon_site/_ro/trn_rl_repo"):
    if os.path.isdir(_p) and _p not in sys.path:
        sys.path.insert(0, _p)

import concourse.bacc as bacc
import concourse.tile as tile
from concourse import mybir
from concourse.bass_utils import run_bass_kernel_spmd

B, S, D = 8, 2048, 128
P = 128          # partitions / token block
CHUNK = 512      # logit chunk width (one PSUM bank of f32)
F32 = mybir.dt.float32
F32R = mybir.dt.float32r
SCALE = float(1.0 / np.sqrt(np.float32(D)))
Copy = mybir.ActivationFunctionType.Copy
Exp = mybir.ActivationFunctionType.Exp


def rnd_f32r(x):
    """Round f32 -> fp32r (zero low 12 mantissa bits, round to nearest)."""
    b = np.ascontiguousarray(x, dtype=np.float32).view(np.uint32).astype(np.uint64)
    b = ((b + 0x800) & 0xFFFFF000).astype(np.uint32)
    return b.view(np.float32)


def build(s=S, d=D, chunk=CHUNK):
    nb = s // P                  # token blocks
    ng = nb // 4                 # groups of 4 blocks (one 512-wide PV chunk)
    nc = bacc.Bacc("TRN2", target_bir_lowering=False, debug=False)
    qT_d = nc.declare_dram_parameter("qT", [P, s], F32R, isOutput=False).ap()
    kT_d = nc.declare_dram_parameter("kT", [P, s], F32R, isOutput=False).ap()
    v_d = nc.declare_dram_parameter("v", [s, d], F32R, isOutput=False).ap()
    jrow_d = nc.declare_dram_parameter("jrow", [2, s], F32R, isOutput=False).ap()
    ineg_d = nc.declare_dram_parameter("ineg", [2, s], F32R, isOutput=False).ap()
    idr_d = nc.declare_dram_parameter("identr", [P, P], F32R, isOutput=False).ap()
    idf_d = nc.declare_dram_parameter("identf", [P, P], F32, isOutput=False).ap()
    tril_d = nc.declare_dram_parameter("tril", [P, P], F32, isOutput=False).ap()
    rep_d = nc.declare_dram_parameter("rep", [P, nb], F32, isOutput=False).ap()
    attn_d = nc.declare_dram_parameter("attn", [s, s], F32, isOutput=True).ap()
    out_d = nc.declare_dram_parameter("out", [s, d], F32, isOutput=True).ap()

    with tile.TileContext(nc) as tc:
        with (
            tc.tile_pool(name="consts", bufs=1) as consts,
            tc.tile_pool(name="big", bufs=1) as big,
            tc.tile_pool(name="e", bufs=3) as epool,
            tc.tile_pool(name="attn", bufs=9) as apool,
            tc.tile_pool(name="strip", bufs=3) as etp,
            tc.tile_pool(name="small", bufs=8) as smp,
            tc.tile_pool(name="oeps", bufs=4) as ops,
            tc.tile_pool(name="psL", bufs=3, space="PSUM") as psL,
            tc.tile_pool(name="psTT", bufs=3, space="PSUM") as psTT,
            tc.tile_pool(name="psTP", bufs=1, space="PSUM") as psTP,
            tc.tile_pool(name="psO", bufs=1, space="PSUM") as psO,
        ):
            # All inputs DMA'd in consumption order: the widest i-blocks
            # (group ng-1) run first, consuming qT/ineg from the BACK and
            # kT/jrow from the FRONT.  Constants the first block needs
            # (tril) lead their queue; bulk v and PV-only constants trail.
            qT = big.tile([P, s], F32R)
            kT = big.tile([P, s], F32R)
            v_t = big.tile([P, nb, d], F32R)
            jrow = consts.tile([P, s], F32R)
            ineg = consts.tile([P, s], F32R)
            tril_t = consts.tile([P, P], F32)
            rep_sb = consts.tile([P, nb], F32)
            ident_r = consts.tile([P, P], F32R)
            ident_f = consts.tile([P, P], F32)
            v_r = v_d.rearrange("(nb p) d -> p nb d", p=P)
            nc.sync.dma_start(out=ident_r[:], in_=idr_d)
            nc.sync.dma_start(out=ident_f[:], in_=idf_d)
            nc.scalar.dma_start(out=tril_t[:], in_=tril_d)
            nc.sync.dma_start(out=rep_sb[:], in_=rep_d)
            nc.vector.memset(jrow[:].bitcast(F32), 0.0)
            nc.sync.dma_start(out=jrow[0:2, :], in_=jrow_d)
            nc.vector.memset(ineg[:].bitcast(F32), 0.0)
            nc.scalar.dma_start(out=ineg[0:2, :], in_=ineg_d)
            nc.sync.dma_start(out=qT[:], in_=qT_d)
            nc.scalar.dma_start(out=kT[:], in_=kT_d)
            nc.sync.dma_start(out=v_t[:], in_=v_r)

            attn_tiles = {}
            pv_psum = [None]
            pv_psum_h = [None]

            def pv_jb(gp, jb, first, last):
                t0 = max(0, jb - 4 * gp)
                strip = etp.tile([P, 512], F32R, tag="s", name="strip")
                if t0 > 0:
                    nc.gpsimd.memset(strip[:, :P * t0].bitcast(F32), 0.0)
                ptt = psTT.tile([P, 512], F32R, tag="tt", name="ptt")
                for t in range(t0, 4):
                    ibx = 4 * gp + t
                    nc.tensor.transpose(
                        ptt[:, P * t:P * (t + 1)],
                        attn_tiles[ibx][:, P * jb:P * (jb + 1)], ident_r[:])
                wc = P * t0
                if jb % 3 != 2:
                    nc.scalar.activation(strip[:, wc:], ptt[:, wc:],
                                         Copy, bias=0.0, scale=1.0)
                else:
                    nc.vector.tensor_copy(strip[:, wc:], ptt[:, wc:])
                nc.tensor.matmul(pv_psum[0][:], v_t[:, jb, :],
                                 strip[:], start=first, stop=last)

            def pv_group_jbs(gp):
                return list(range(4 * gp + 4))

            def pv_epilogue(gp):
                po = pv_psum[0]
                oT = ops.tile([P, 512], F32, tag="ot", name="oT")
                nc.scalar.activation(oT[:], po[:], Copy, bias=0.0, scale=1.0)
                for t in range(4):
                    pt3 = psTP.tile([P, P], F32, tag="tp", name="pt3")
                    nc.tensor.transpose(pt3[:], oT[:, P * t:P * (t + 1)],
                                        ident_f[:])
                    o_sb = ops.tile([P, d], F32, tag="os", name="o_sb")
                    nc.scalar.activation(o_sb[:], pt3[:], Copy, bias=0.0,
                                         scale=1.0)
                    ibx = 4 * gp + t
                    nc.sync.dma_start(out=out_d[P * ibx:P * (ibx + 1), :],
                                      in_=o_sb[:])

            def phase_a(ib):
                width = P * (ib + 1)
                nch = (width + chunk - 1) // chunk
                e_t = epool.tile([P, s], F32, tag="e", name="e_t")
                zacc = smp.tile([P, max(s // chunk, 1)], F32, tag="z",
                                name="zacc")
                for c in range(nch):
                    j0 = c * chunk
                    w = min(chunk, width - j0)
                    ps = psL.tile([P, chunk], F32, tag="L", name="ps")
                    nc.tensor.matmul(ps[:, :w], qT[:, P * ib:P * (ib + 1)],
                                     kT[:, j0:j0 + w], start=True, stop=False)
                    nc.tensor.matmul(ps[:, :w], ineg[:, P * ib:P * (ib + 1)],
                                     jrow[:, j0:j0 + w], start=False, stop=True)
                    last = c == nch - 1
                    if last:
                        # mask the diagonal block's logits (cols >= i
                        # in-block would otherwise exp-overflow)
                        dc = P * ib - j0
                        nc.vector.tensor_mul(ps[:, dc:dc + P],
                                             ps[:, dc:dc + P], tril_t[:])
                    nc.scalar.activation(
                        e_t[:, j0:j0 + w], ps[:, :w], Exp, bias=0.0, scale=1.0,
                        accum_out=None if last else zacc[:, c:c + 1])
                # strict-tril zero of the diagonal block (masked exp(0)=1
                # entries must not reach Z or the PV transposes)
                nc.vector.tensor_mul(e_t[:, P * ib:width],
                                     e_t[:, P * ib:width], tril_t[:])
                j0l = (nch - 1) * chunk
                nc.vector.tensor_reduce(
                    zacc[:, nch - 1:nch], e_t[:, j0l:width],
                    axis=mybir.AxisListType.X, op=mybir.AluOpType.add)
                z = smp.tile([P, 1], F32, tag="zz", name="z")
                nc.vector.tensor_reduce(
                    z[:], zacc[:, :nch],
                    axis=mybir.AxisListType.X, op=mybir.AluOpType.add)
                nc.vector.tensor_scalar_add(z[:], z[:], 1e-20)
                rc = smp.tile([P, 1], F32, tag="rc", name="rc")
                nc.vector.reciprocal(rc[:], z[:])
                nc.vector.tensor_mul(rc[:], rc[:], rep_sb[:, ib:ib + 1])
                a_t = apool.tile([P, s], F32R, tag="a", name="a_t")
                nc.vector.tensor_scalar_mul(a_t[:, :width], e_t[:, :width],
                                            rc[:])
                eng = nc.sync if ib % 2 == 0 else nc.scalar
                eng.dma_start(out=attn_d[P * ib:P * (ib + 1), 0:width],
                              in_=a_t[:, :width].bitcast(F32))
                attn_tiles[ib] = a_t

            def pv_half(gp, half, jbs_part, first_jb, last_jb):
                # half 0: strip/psum columns [0:256] = blocks 4gp,4gp+1;
                # half 1: columns [256:512] = blocks 4gp+2,4gp+3
                c0, c1 = (0, 256) if half == 0 else (256, 512)
                for jb in jbs_part:
                    strip_f = etp.tile([P, 512], F32R, tag="s", name="striph"); strip = strip_f[:, :256]
                    ptt_f = psTT.tile([P, 512], F32R, tag="tt", name="ptth"); ptt = ptt_f[:, :256]
                    done_memset = False
                    for t in range(2):
                        ibx = 4 * gp + 2 * half + t
                        if jb > ibx:  # fully valid or diag
                            nc.tensor.transpose(
                                ptt[:, P * t:P * (t + 1)],
                                attn_tiles[ibx][:, P * jb:P * (jb + 1)],
                                ident_r[:]) if False else None
                        if ibx < jb:
                            if not done_memset:
                                nc.vector.memset(
                                    strip[:, P * t:P * (t + 1)].bitcast(F32),
                                    0.0)
                        else:
                            nc.tensor.transpose(
                                ptt[:, P * t:P * (t + 1)],
                                attn_tiles[ibx][:, P * jb:P * (jb + 1)],
                                ident_r[:])
                    t0 = max(0, jb - (4 * gp + 2 * half))
                    wc = P * min(t0, 2)
                    if wc < 256:
                        if jb % 3 != 2:
                            nc.scalar.activation(strip[:, wc:], ptt[:, wc:],
                                                 Copy, bias=0.0, scale=1.0)
                        else:
                            nc.vector.tensor_copy(strip[:, wc:], ptt[:, wc:])
                    nc.tensor.matmul(pv_psum_h[0], v_t[:, jb, :],
                                     strip[:], start=(jb == first_jb),
                                     stop=(jb == last_jb))

            def pv_half_epilogue(gp, half):
                po = pv_psum_h[0]
                oT_f = ops.tile([P, 512], F32, tag="ot", name="oTh")
                oT = oT_f[:, :256]
                nc.vector.tensor_copy(oT, po)
                for t in range(2):
                    pt3 = psTP.tile([P, P], F32, tag="tp", name="pt3")
                    nc.tensor.transpose(pt3[:], oT[:, P * t:P * (t + 1)],
                                        ident_f[:])
                    o_sb = ops.tile([P, d], F32, tag="os", name="o_sb")
                    nc.scalar.activation(o_sb[:], pt3[:], Copy, bias=0.0,
                                         scale=1.0)
                    ibx = 4 * gp + 2 * half + t
                    nc.sync.dma_start(out=out_d[P * ibx:P * (ibx + 1), :],
                                      in_=o_sb[:])

            groups = list(range(ng))  # ascending: small bare group first
            glast = ng - 1
            jbs_l = list(range(4 * glast + 2))   # left half: j < blocks 0..4g+1
            for si, g in enumerate(groups):
                prev = groups[si - 1] if si > 0 else None
                parts = [[] for _ in range(4)]
                if prev is not None:
                    jbs = pv_group_jbs(prev)
                    for i_, jb in enumerate(jbs):
                        parts[i_ * 4 // len(jbs)].append(jb)
                for t4 in range(4):
                    phase_a(4 * g + t4)
                    if prev is not None:
                        if t4 == 0:
                            po_t = psO.tile([P, 512], F32, tag="o", name="po")
                            pv_psum[0] = po_t
                        jbs = pv_group_jbs(prev)
                        for jb in parts[t4]:
                            pv_jb(prev, jb, first=(jb == jbs[0]),
                                  last=(jb == jbs[-1]))
                        if t4 == 3:
                            pv_epilogue(prev)
                    # left half of the last group's PV: ready after
                    # phaseA(4*glast+1), emitted once the previous group's
                    # PV released its PSUM slot
                    if g == glast and t4 == 3:
                        po_t = psO.tile([P, 512], F32, tag="o", name="poh")
                        pv_psum_h[0] = po_t[:, :256]
                        for jb in jbs_l:
                            pv_half(glast, 0, [jb], jbs_l[0], jbs_l[-1])
                        pv_half_epilogue(glast, 0)

            # drain: right half of the last group's PV
            gp = glast
            po_t = psO.tile([P, 512], F32, tag="o", name="poh")
            pv_psum_h[0] = po_t[:, :256]
            jbs_r = list(range(4 * gp + 4))
            for jb in jbs_r:
                pv_half(gp, 1, [jb], jbs_r[0], jbs_r[-1])
            pv_half_epilogue(gp, 1)

    nc.compile()
    return nc


_NC_CACHE = {}
LAST_RESULT = None  # BassKernelResults of the most recent kernel() call


def _get_nc():
    if "nc" not in _NC_CACHE:
        _NC_CACHE["nc"] = build()
    return _NC_CACHE["nc"]


def _consts():
    if "consts" not in _NC_CACHE:
        j = np.arange(S, dtype=np.float32)
        jrow = np.stack([np.ones(S, np.float32), j])          # [2, S]
        ineg = np.stack([-j, np.ones(S, np.float32)])         # [2, S]
        ident = np.eye(P, dtype=np.float32)
        tril = np.tril(np.ones((P, P), np.float32), k=-1)
        _NC_CACHE["consts"] = (jrow, ineg, ident, tril)
    return _NC_CACHE["consts"]


def kernel(q, k, v, rep_mask):
    nc = _get_nc()
    jrow, ineg, ident, tril = _consts()
    in_maps = []
    for b in range(B):
        in_maps.append({
            "qT": rnd_f32r(np.ascontiguousarray(q[b].T) * np.float32(SCALE)),
            "kT": rnd_f32r(np.ascontiguousarray(k[b].T)),
            "v": rnd_f32r(v[b]),
            "jrow": jrow,
            "ineg": ineg,
            "identr": ident,
            "identf": ident,
            "tril": tril,
            "rep": np.ascontiguousarray(
                rep_mask[b].reshape(S // P, P).T.astype(np.float32)),
        })
    res = run_bass_kernel_spmd(nc, in_maps, core_ids=list(range(B)))
    global LAST_RESULT
    LAST_RESULT = res
    out = np.stack([res.results[b]["out"] for b in range(B)])
    attn = np.stack([res.results[b]["attn"] for b in range(B)])
    return out, attn


# revision 56
# speedup vs baseline: 1.0586x; 1.0586x over previous
"""Trainium2 Bass kernel: batched masked sparse attention.

Problem (per batch element b of 8):
    logits = q @ k.T / sqrt(D) - |i - j|                    [S, S]
    mask   = rep_i * rep_j * strict_lower_tri               (rep = first-L valid)
    attn   = masked_softmax(logits, mask)  (mask-multiply semantics)
    out    = attn @ v
Returns (out [B,S,D], attn [B,S,S]).

Strategy: batch-parallel SPMD — one batch element per NeuronCore, 8 cores,
no collectives.  Per core:
  * Only the strictly-lower-triangular region of attn is ever nonzero
    (valid rows i < L imply j < i < L, so the column mask is redundant);
    the upper triangle is left to the runtime's zero-initialized output.
  * No row-max subtraction: valid logits <= qk/sqrt(D) (distance bias is
    negative there), exp can't overflow, and softmax is shift-invariant,
    so e = exp(logits) directly; row sums come free from the ACT
    accumulator.  Invalid rows are zeroed via recip = rep_i/(Z + 1e-20).
  * QK^T and the -|i-j| distance bias are both TensorE matmuls (fp32r
    fast path, 1 cycle/row; the bias via a K=128 zero-padded matmul
    against iota operands) accumulated in PSUM.
  * PV runs on PE-transposed attn blocks: out^T[d, i] accumulates
    v_j^T @ attn^T[j, i] over j, then a final PE transpose restores [i,d].
  * Software pipelining: each 4-block group's PV work is interleaved
    into the NEXT group's phaseA emission (per-engine execution is
    in-order, so this keeps TensorE from stalling on the softmax chain);
    the final group's PV is split into two half-width accumulations so
    half of it overlaps the last phaseA blocks.
  * Host-side prep inside kernel(): q/k are pre-transposed and, like v,
    pre-rounded to fp32r (11 mantissa bits); iota/identity/tril constant
    tiles are shipped as extra NEFF inputs.  GpSimd is kept out of the
    pipeline entirely (its SBUF port is shared with DVE).
"""

import os
import sys

import numpy as np

for _p in ("/opt/trn_rl_repo", "/root/.axon_site/_ro/trn_rl_repo"):
    if os.path.isdir(_p) and _p not in sys.path:
        sys.path.insert(0, _p)

import concourse.bacc as bacc
import concourse.tile as tile
from concourse import mybir
from concourse.bass_utils import run_bass_kernel_spmd

B, S, D = 8, 2048, 128
P = 128          # partitions / token block
CHUNK = 512      # logit chunk width (one PSUM bank of f32)
F32 = mybir.dt.float32
F32R = mybir.dt.float32r
SCALE = float(1.0 / np.sqrt(np.float32(D)))
Copy = mybir.ActivationFunctionType.Copy
Exp = mybir.ActivationFunctionType.Exp


def rnd_f32r(x):
    """Round f32 -> fp32r (zero low 12 mantissa bits, round to nearest)."""
    b = np.ascontiguousarray(x, dtype=np.float32).view(np.uint32).astype(np.uint64)
    b = ((b + 0x800) & 0xFFFFF000).astype(np.uint32)
    return b.view(np.float32)


def build(s=S, d=D, chunk=CHUNK):
    nb = s // P                  # token blocks
    ng = nb // 4                 # groups of 4 blocks (one 512-wide PV chunk)
    nc = bacc.Bacc("TRN2", target_bir_lowering=False, debug=False)
    qT_d = nc.declare_dram_parameter("qT", [P, s], F32R, isOutput=False).ap()
    kT_d = nc.declare_dram_parameter("kT", [P, s], F32R, isOutput=False).ap()
    v_d = nc.declare_dram_parameter("v", [s, d], F32R, isOutput=False).ap()
    jrow_d = nc.declare_dram_parameter("jrow", [2, s], F32R, isOutput=False).ap()
    ineg_d = nc.declare_dram_parameter("ineg", [2, s], F32R, isOutput=False).ap()
    idr_d = nc.declare_dram_parameter("identr", [P, P], F32R, isOutput=False).ap()
    idf_d = nc.declare_dram_parameter("identf", [P, P], F32, isOutput=False).ap()
    tril_d = nc.declare_dram_parameter("tril", [P, P], F32, isOutput=False).ap()
    rep_d = nc.declare_dram_parameter("rep", [P, nb], F32, isOutput=False).ap()
    attn_d = nc.declare_dram_parameter("attn", [s, s], F32, isOutput=True).ap()
    out_d = nc.declare_dram_parameter("out", [s, d], F32, isOutput=True).ap()

    with tile.TileContext(nc) as tc:
        with (
            tc.tile_pool(name="consts", bufs=1) as consts,
            tc.tile_pool(name="big", bufs=1) as big,
            tc.tile_pool(name="e", bufs=3) as epool,
            tc.tile_pool(name="attn", bufs=9) as apool,
            tc.tile_pool(name="strip", bufs=3) as etp,
            tc.tile_pool(name="small", bufs=8) as smp,
            tc.tile_pool(name="oeps", bufs=4) as ops,
            tc.tile_pool(name="psL", bufs=3, space="PSUM") as psL,
            tc.tile_pool(name="psTT", bufs=3, space="PSUM") as psTT,
            tc.tile_pool(name="psTP", bufs=1, space="PSUM") as psTP,
            tc.tile_pool(name="psO", bufs=1, space="PSUM") as psO,
        ):
            # All inputs DMA'd in consumption order: the widest i-blocks
            # (group ng-1) run first, consuming qT/ineg from the BACK and
            # kT/jrow from the FRONT.  Constants the first block needs
            # (tril) lead their queue; bulk v and PV-only constants trail.
            qT = big.tile([P, s], F32R)
            kT = big.tile([P, s], F32R)
            v_t = big.tile([P, nb, d], F32R)
            jrow = consts.tile([P, s], F32R)
            ineg = consts.tile([P, s], F32R)
            tril_t = consts.tile([P, P], F32)
            rep_sb = consts.tile([P, nb], F32)
            ident_r = consts.tile([P, P], F32R)
            ident_f = consts.tile([P, P], F32)
            v_r = v_d.rearrange("(nb p) d -> p nb d", p=P)
            nc.sync.dma_start(out=ident_r[:], in_=idr_d)
            nc.sync.dma_start(out=ident_f[:], in_=idf_d)
            nc.scalar.dma_start(out=tril_t[:], in_=tril_d)
            nc.sync.dma_start(out=rep_sb[:], in_=rep_d)
            nc.vector.memset(jrow[:].bitcast(F32), 0.0)
            nc.sync.dma_start(out=jrow[0:2, :], in_=jrow_d)
            nc.vector.memset(ineg[:].bitcast(F32), 0.0)
            nc.scalar.dma_start(out=ineg[0:2, :], in_=ineg_d)
            nc.sync.dma_start(out=qT[:], in_=qT_d)
            nc.scalar.dma_start(out=kT[:], in_=kT_d)
            nc.sync.dma_start(out=v_t[:], in_=v_r)

            attn_tiles = {}
            pv_psum = [None]
            pv_psum_h = [None]

            def pv_jb(gp, jb, first, last):
                t0 = max(0, jb - 4 * gp)
                strip = etp.tile([P, 512], F32R, tag="s", name="strip")
                if t0 > 0:
                    nc.vector.memset(strip[:, :P * t0].bitcast(F32), 0.0)
                ptt = psTT.tile([P, 512], F32R, tag="tt", name="ptt")
                for t in range(t0, 4):
                    ibx = 4 * gp + t
                    nc.tensor.transpose(
                        ptt[:, P * t:P * (t + 1)],
                        attn_tiles[ibx][:, P * jb:P * (jb + 1)], ident_r[:])
                wc = P * t0
                if jb % 3 != 2:
                    nc.scalar.activation(strip[:, wc:], ptt[:, wc:],
                                         Copy, bias=0.0, scale=1.0)
                else:
                    nc.vector.tensor_copy(strip[:, wc:], ptt[:, wc:])
                nc.tensor.matmul(pv_psum[0][:], v_t[:, jb, :],
                                 strip[:], start=first, stop=last)

            def pv_group_jbs(gp):
                return list(range(4 * gp + 4))

            def pv_epilogue(gp):
                po = pv_psum[0]
                oT = ops.tile([P, 512], F32, tag="ot", name="oT")
                nc.scalar.activation(oT[:], po[:], Copy, bias=0.0, scale=1.0)
                for t in range(4):
                    pt3 = psTP.tile([P, P], F32, tag="tp", name="pt3")
                    nc.tensor.transpose(pt3[:], oT[:, P * t:P * (t + 1)],
                                        ident_f[:])
                    o_sb = ops.tile([P, d], F32, tag="os", name="o_sb")
                    nc.scalar.activation(o_sb[:], pt3[:], Copy, bias=0.0,
                                         scale=1.0)
                    ibx = 4 * gp + t
                    nc.sync.dma_start(out=out_d[P * ibx:P * (ibx + 1), :],
                                      in_=o_sb[:])

            def phase_a(ib):
                width = P * (ib + 1)
                nch = (width + chunk - 1) // chunk
                e_t = epool.tile([P, s], F32, tag="e", name="e_t")
                zacc = smp.tile([P, max(s // chunk, 1)], F32, tag="z",
                                name="zacc")
                for c in range(nch):
                    j0 = c * chunk
                    w = min(chunk, width - j0)
                    ps = psL.tile([P, chunk], F32, tag="L", name="ps")
                    nc.tensor.matmul(ps[:, :w], qT[:, P * ib:P * (ib + 1)],
                                     kT[:, j0:j0 + w], start=True, stop=False)
                    nc.tensor.matmul(ps[:, :w], ineg[:, P * ib:P * (ib + 1)],
                                     jrow[:, j0:j0 + w], start=False, stop=True)
                    last = c == nch - 1
                    if last:
                        # mask the diagonal block's logits (cols >= i
                        # in-block would otherwise exp-overflow)
                        dc = P * ib - j0
                        nc.vector.tensor_mul(ps[:, dc:dc + P],
                                             ps[:, dc:dc + P], tril_t[:])
                    nc.scalar.activation(
                        e_t[:, j0:j0 + w], ps[:, :w], Exp, bias=0.0, scale=1.0,
                        accum_out=None if last else zacc[:, c:c + 1])
                # strict-tril zero of the diagonal block (masked exp(0)=1
                # entries must not reach Z or the PV transposes)
                nc.vector.tensor_mul(e_t[:, P * ib:width],
                                     e_t[:, P * ib:width], tril_t[:])
                j0l = (nch - 1) * chunk
                nc.vector.tensor_reduce(
                    zacc[:, nch - 1:nch], e_t[:, j0l:width],
                    axis=mybir.AxisListType.X, op=mybir.AluOpType.add)
                z = smp.tile([P, 1], F32, tag="zz", name="z")
                nc.vector.tensor_reduce(
                    z[:], zacc[:, :nch],
                    axis=mybir.AxisListType.X, op=mybir.AluOpType.add)
                nc.vector.tensor_scalar_add(z[:], z[:], 1e-20)
                rc = smp.tile([P, 1], F32, tag="rc", name="rc")
                nc.vector.reciprocal(rc[:], z[:])
                nc.vector.tensor_mul(rc[:], rc[:], rep_sb[:, ib:ib + 1])
                a_t = apool.tile([P, s], F32R, tag="a", name="a_t")
                nc.vector.tensor_scalar_mul(a_t[:, :width], e_t[:, :width],
                                            rc[:])
                eng = nc.sync if ib % 2 == 0 else nc.scalar
                eng.dma_start(out=attn_d[P * ib:P * (ib + 1), 0:width],
                              in_=a_t[:, :width].bitcast(F32))
                attn_tiles[ib] = a_t

            def pv_half(gp, half, jbs_part, first_jb, last_jb):
                # half 0: strip/psum columns [0:256] = blocks 4gp,4gp+1;
                # half 1: columns [256:512] = blocks 4gp+2,4gp+3
                c0, c1 = (0, 256) if half == 0 else (256, 512)
                for jb in jbs_part:
                    strip_f = etp.tile([P, 512], F32R, tag="s", name="striph"); strip = strip_f[:, :256]
                    ptt_f = psTT.tile([P, 512], F32R, tag="tt", name="ptth"); ptt = ptt_f[:, :256]
                    done_memset = False
                    for t in range(2):
                        ibx = 4 * gp + 2 * half + t
                        if jb > ibx:  # fully valid or diag
                            nc.tensor.transpose(
                                ptt[:, P * t:P * (t + 1)],
                                attn_tiles[ibx][:, P * jb:P * (jb + 1)],
                                ident_r[:]) if False else None
                        if ibx < jb:
                            if not done_memset:
                                nc.vector.memset(
                                    strip[:, P * t:P * (t + 1)].bitcast(F32),
                                    0.0)
                        else:
                            nc.tensor.transpose(
                                ptt[:, P * t:P * (t + 1)],
                                attn_tiles[ibx][:, P * jb:P * (jb + 1)],
                                ident_r[:])
                    t0 = max(0, jb - (4 * gp + 2 * half))
                    wc = P * min(t0, 2)
                    if wc < 256:
                        if jb % 3 != 2:
                            nc.scalar.activation(strip[:, wc:], ptt[:, wc:],
                                                 Copy, bias=0.0, scale=1.0)
                        else:
                            nc.vector.tensor_copy(strip[:, wc:], ptt[:, wc:])
                    nc.tensor.matmul(pv_psum_h[0], v_t[:, jb, :],
                                     strip[:], start=(jb == first_jb),
                                     stop=(jb == last_jb))

            def pv_half_epilogue(gp, half):
                po = pv_psum_h[0]
                oT_f = ops.tile([P, 512], F32, tag="ot", name="oTh")
                oT = oT_f[:, :256]
                nc.vector.tensor_copy(oT, po)
                for t in range(2):
                    pt3 = psTP.tile([P, P], F32, tag="tp", name="pt3")
                    nc.tensor.transpose(pt3[:], oT[:, P * t:P * (t + 1)],
                                        ident_f[:])
                    o_sb = ops.tile([P, d], F32, tag="os", name="o_sb")
                    nc.scalar.activation(o_sb[:], pt3[:], Copy, bias=0.0,
                                         scale=1.0)
                    ibx = 4 * gp + 2 * half + t
                    nc.sync.dma_start(out=out_d[P * ibx:P * (ibx + 1), :],
                                      in_=o_sb[:])

            groups = list(range(ng))  # ascending: small bare group first
            glast = ng - 1
            jbs_l = list(range(4 * glast + 2))   # left half: j < blocks 0..4g+1
            for si, g in enumerate(groups):
                prev = groups[si - 1] if si > 0 else None
                parts = [[] for _ in range(4)]
                if prev is not None:
                    jbs = pv_group_jbs(prev)
                    for i_, jb in enumerate(jbs):
                        parts[i_ * 4 // len(jbs)].append(jb)
                for t4 in range(4):
                    phase_a(4 * g + t4)
                    if prev is not None:
                        if t4 == 0:
                            po_t = psO.tile([P, 512], F32, tag="o", name="po")
                            pv_psum[0] = po_t
                        jbs = pv_group_jbs(prev)
                        for jb in parts[t4]:
                            pv_jb(prev, jb, first=(jb == jbs[0]),
                                  last=(jb == jbs[-1]))
                        if t4 == 3:
                            pv_epilogue(prev)
                    # left half of the last group's PV: ready after
                    # phaseA(4*glast+1), emitted once the previous group's
                    # PV released its PSUM slot
                    if g == glast and t4 == 3:
                        po_t = psO.tile([P, 512], F32, tag="o", name="poh")
                        pv_psum_h[0] = po_t[:, :256]
                        for jb in jbs_l:
                            pv_half(glast, 0, [jb], jbs_l[0], jbs_l[-1])
                        pv_half_epilogue(glast, 0)

            # drain: right half of the last group's PV
            gp = glast
            po_t = psO.tile([P, 512], F32, tag="o", name="poh")
            pv_psum_h[0] = po_t[:, :256]
            jbs_r = list(range(4 * gp + 4))
            for jb in jbs_r:
                pv_half(gp, 1, [jb], jbs_r[0], jbs_r[-1])
            pv_half_epilogue(gp, 1)

    nc.compile()
    return nc


_NC_CACHE = {}
LAST_RESULT = None  # BassKernelResults of the most recent kernel() call


def _get_nc():
    if "nc" not in _NC_CACHE:
        _NC_CACHE["nc"] = build()
    return _NC_CACHE["nc"]


def _consts():
    if "consts" not in _NC_CACHE:
        j = np.arange(S, dtype=np.float32)
        jrow = np.stack([np.ones(S, np.float32), j])          # [2, S]
        ineg = np.stack([-j, np.ones(S, np.float32)])         # [2, S]
        ident = np.eye(P, dtype=np.float32)
        tril = np.tril(np.ones((P, P), np.float32), k=-1)
        _NC_CACHE["consts"] = (jrow, ineg, ident, tril)
    return _NC_CACHE["consts"]


def kernel(q, k, v, rep_mask):
    nc = _get_nc()
    jrow, ineg, ident, tril = _consts()
    in_maps = []
    for b in range(B):
        in_maps.append({
            "qT": rnd_f32r(np.ascontiguousarray(q[b].T) * np.float32(SCALE)),
            "kT": rnd_f32r(np.ascontiguousarray(k[b].T)),
            "v": rnd_f32r(v[b]),
            "jrow": jrow,
            "ineg": ineg,
            "identr": ident,
            "identf": ident,
            "tril": tril,
            "rep": np.ascontiguousarray(
                rep_mask[b].reshape(S // P, P).T.astype(np.float32)),
        })
    res = run_bass_kernel_spmd(nc, in_maps, core_ids=list(range(B)))
    global LAST_RESULT
    LAST_RESULT = res
    out = np.stack([res.results[b]["out"] for b in range(B)])
    attn = np.stack([res.results[b]["attn"] for b in range(B)])
    return out, attn
